# revision 68
# baseline (speedup 1.0000x reference)
"""CrystalGraphConvNet Bass/Tile kernel for TRN2 (8-core data-parallel).

Device algorithm (per core, 2 crystals, BJ=192 bj-rows, R=18432 (bj,k)-rows):
  - gated = conv(total) computed as ONE augmented bf16 matmul per row-block:
      lhsT [128, 64] = [W3 ; 0 ; A'^T_block ; B^T_block], rhs_pack [128, cols] =
      [nbrT ; 0 ; ones-diag ; adj-diag]  -> raw gated in PSUM, partition-packed
      (filt(H0)/filt(H1) stacked to use all 128 lanes downstream).
  - bn1 stats computed analytically (no pass over gated): host supplies
    layer-independent nbr/adj reductions; device computes fea-dependent
    terms TRANSPOSED (aux [ones|deg] as 2-col LDWEIGHTS, data as rhs) so
    the collective payload is [2,896] (2 DMA packets, not 128x8B = 16
    serialized packets); AllGather + a ones/sel16-weighted PE matmul does
    the 8-core reduce, transpose-back to channel-major, AND the bn1
    mean/E[x^2] weighting in one step.
  - sigmoid via ACT Sigmoid table (bn1 affine as per-partition scale/bias
    from PSUM); core half drained on DVE with the NEGATED affine folded in
    (host negates core-half bn1 g/b and bn2 gain), so softplus is
    -Ln(Sigmoid(-z)) and stays in two ACT tables; chunk Sigmoids batched
    before Lns to minimize table loads.
  - h = sig*sp on DVE; k-sum via contiguous-halves add tree (bf16 2x).
  - bn2: free-dim reduce + PE transpose to a [1,256] single-packet payload,
    AllGather + ones-matmul reduce back to [64,2]; fea update via softplus.
"""

import numpy as np
import ml_dtypes

import concourse.bass as bass
import concourse.mybir as mybir
from concourse import tile

F32 = mybir.dt.float32
BF16 = mybir.dt.bfloat16
FP8 = mybir.dt.float8e4
I32 = mybir.dt.int32
AF = mybir.ActivationFunctionType
OP = mybir.AluOpType

EPS = 1e-5
N0, N, ORIG, F, K, H, NC = 16, 96, 92, 64, 41, 128, 3
NCORES, BPC = 8, 2
BJ = BPC * N            # 192
R = BJ * N              # 18432
G32 = 32
NBLK = BJ // G32        # 6
HALF = R // 2           # 9216
NTOT = float(N0 * N * N)
NTOT2 = float(N0 * N)
NGRP = 18               # main groups per layer, 512 paired-cols each
GW = 512
SPCH = 4                # softplus/mul/tree chunks
CHW = HALF // SPCH      # 2304 = 24 bj * 96


def bf16(x):
    return np.ascontiguousarray(np.asarray(x, np.float32).astype(ml_dtypes.bfloat16))


def fp8(x):
    return np.ascontiguousarray(
        np.asarray(x, np.float32).astype(ml_dtypes.float8_e4m3fn))


INPUT_SPECS = [
    ("rhs_pack", (128, R), BF16),
    ("atomT", (ORIG + 1, BJ), F32),
    ("emb", (ORIG + 1, F), F32),
    ("w3", (K, NC * 128), BF16),
    ("wab", (F + 1, NC * 256), BF16),
    ("s1s", (G32, NC * NBLK * 256), BF16),
    ("aux", (G32, 2 * NBLK), BF16),
    ("sel16", (2 * NCORES, 4), F32),
    ("ident", (128, 128), F32),
    ("gvec", (128, 12), F32),
    ("gvec2", (F, 6), F32),
    ("fcW", (F, H), F32),
    ("fcb", (H, 1), F32),
    ("outW", (H, 1), F32),
    ("outb", (1, 1), F32),
]


def host_prep(inputs):
    """Build the 8 per-core input maps from the full problem inputs."""
    atom_fea = np.asarray(inputs["atom_fea"], np.float32)
    nbr_fea = np.asarray(inputs["nbr_fea"], np.float32)
    adj = np.asarray(inputs["adj"])
    conv_W = np.asarray(inputs["conv_W"], np.float64)
    conv_b = np.asarray(inputs["conv_b"], np.float64)

    emb_ext = np.concatenate(
        [np.asarray(inputs["emb_W"], np.float32),
         np.asarray(inputs["emb_b"], np.float32)[None]], 0)
    w3_all = np.concatenate([bf16(conv_W[l, 2 * F:]) for l in range(NC)], 1)
    wab_all = bf16(np.concatenate(
        [np.concatenate(
            [np.concatenate([conv_W[l, :F], conv_b[l][None]], 0),
             np.concatenate([conv_W[l, F:2 * F], np.zeros((1, 2 * F))], 0)], 1)
         for l in range(NC)], 1))
    fcW = np.asarray(inputs["fc_W"], np.float32)
    # negated: consumed as the Sigmoid nbias inside _softplus (see kernel)
    fcb = -np.asarray(inputs["fc_b"], np.float32).reshape(H, 1)
    outW = np.asarray(inputs["out_W"], np.float32).reshape(H, 1)
    outb = np.asarray(inputs["out_b"], np.float32).reshape(1, 1)
    bn1_g = np.asarray(inputs["bn1_g"], np.float32)
    bn1_b = np.asarray(inputs["bn1_b"], np.float32)
    bn2_g = np.asarray(inputs["bn2_g"], np.float32)
    bn2_b = np.asarray(inputs["bn2_b"], np.float32)

    colbj = np.arange(R) // N
    gidx = colbj % G32

    per_core, nbrsum_g, gram_g = [], 0.0, 0.0
    for c in range(NCORES):
        sl = slice(c * BPC, (c + 1) * BPC)
        nbr = nbr_fea[sl].reshape(R, K).astype(np.float64)
        adjf = adj[sl].reshape(R).astype(np.float64)
        deg = adjf.reshape(BJ, N).sum(1)
        rhs = np.zeros((128, R), np.float32)
        rhs[0:K] = nbr.T
        rhs[64 + gidx, np.arange(R)] = 1.0
        rhs[96 + gidx, np.arange(R)] = adjf
        nbrj = nbr.reshape(BJ, N, K).sum(1)
        nbrja = (nbr.reshape(BJ, N, K) * adjf.reshape(BJ, N, 1)).sum(1)
        s1s = np.empty((G32, NC * NBLK * 256), np.float64)
        for l in range(NC):
            W3 = conv_W[l, 2 * F:]
            S1T, S1aT = nbrj @ W3, nbrja @ W3
            for b in range(NBLK):
                blk = np.concatenate(
                    [S1T[b * G32:(b + 1) * G32], S1aT[b * G32:(b + 1) * G32]], 1)
                s1s[:, (l * NBLK + b) * 256:(l * NBLK + b + 1) * 256] = blk
        # per block b: col 2b = ones, col 2b+1 = deg (stat-matmul weights)
        aux = np.zeros((G32, 2 * NBLK), np.float64)
        for b in range(NBLK):
            aux[:, 2 * b] = 1.0
            aux[:, 2 * b + 1] = deg[b * G32:(b + 1) * G32]
        atomT = np.concatenate(
            [atom_fea[sl].reshape(BJ, ORIG).T, np.ones((1, BJ))], 0).astype(np.float32)
        nbrsum_g = nbrsum_g + nbr.sum(0)
        gram_g = gram_g + nbr.T @ nbr
        per_core.append(dict(rhs=bf16(rhs), atomT=atomT, s1s=bf16(s1s), aux=bf16(aux)))

    # Core-half bn1 params and bn2 gain are negated host-side: the kernel
    # computes softplus(z) as -Ln(Sigmoid(-z)), so the core affine must
    # produce -z, and the resulting negated `summed` is fixed up in bn2 by
    # the negated gain (bias formula is sign-invariant).
    # cols 0:3 / 3:6 are pre-divided by NTOT: the AR1 reduce-matmuls also
    # fold the (N, 1, 2)/NTOT stat weights (sel16), so they produce the bn1
    # mean / E[x^2] directly.
    gvec = np.zeros((128, 12), np.float32)
    for l in range(NC):
        W3 = conv_W[l, 2 * F:]
        gvec[:, l] = (nbrsum_g @ W3) / NTOT
        gvec[:, 3 + l] = np.einsum("fc,fg,gc->c", W3, gram_g, W3) / NTOT
        gvec[:, 6 + l] = bn1_g[l]
        gvec[F:128, 6 + l] *= -1.0
        gvec[:, 9 + l] = bn1_b[l]
        gvec[F:128, 9 + l] *= -1.0
    gvec2 = np.zeros((F, 6), np.float32)
    for l in range(NC):
        gvec2[:, l] = -bn2_g[l]
        gvec2[:, 3 + l] = bn2_b[l]

    sel16 = np.zeros((2 * NCORES, 4), np.float32)
    sel16[0::2, 0] = N / NTOT
    sel16[1::2, 1] = 1.0 / NTOT
    sel16[0::2, 2] = 2.0 / NTOT
    sel16[1::2, 3] = 2.0 / NTOT

    in_maps = []
    for c in range(NCORES):
        pc = per_core[c]
        in_maps.append({
            "rhs_pack": pc["rhs"], "atomT": pc["atomT"], "emb": emb_ext,
            "w3": w3_all, "wab": wab_all, "s1s": pc["s1s"], "aux": pc["aux"],
            "ident": np.eye(128, dtype=np.float32), "sel16": sel16,
            "gvec": gvec, "gvec2": gvec2, "fcW": fcW, "fcb": fcb,
            "outW": outW, "outb": outb,
        })
    return in_maps


def _softplus(nc, pool, out, in_, tag, nbias=0.0, nscale=-1.0):
    """out = softplus(x) = Ln(Exp(x) + 1); pass nscale=-scale, nbias=-bias
    (negated args kept for call-site compatibility; Exp flips them back).

    Exp and the chunk-loop Lns can share the natural_log_exp table set,
    so this costs no extra ACT table loads."""
    p, fd = out.shape[0], int(np.prod(out.shape[1:]))
    e = pool.tile([p, fd], F32, tag=tag + "_e")
    if isinstance(nbias, float):
        nc.scalar.activation(e[:], in_, AF.Exp, bias=-nbias, scale=-nscale)
    else:
        nb = pool.tile([p, 1], F32, tag=tag + "_nb")
        nc.vector.tensor_scalar(nb[:], nbias, -1.0, None, OP.mult)
        nc.scalar.activation(e[:], in_, AF.Exp, bias=nb[:], scale=-nscale)
    nc.scalar.activation(out, e[:], AF.Ln, bias=1.0)


def _rsqrt(nc, pool, out, v, p):
    """out = 1/sqrt(v), [p,1] f32, via magic-init + 3 Newton iterations."""
    yb = pool.tile([p, 1], I32, tag="rs_i")
    nc.vector.tensor_scalar(yb[:], v.bitcast(I32), 1, None, OP.logical_shift_right)
    nc.vector.tensor_scalar(yb[:], yb[:], -1, 0x5F3759DF, OP.mult, OP.add)
    y = yb.bitcast(F32)
    t = pool.tile([p, 1], F32, tag="rs_t")
    for _ in range(2):
        nc.vector.tensor_tensor(t[:], y[:], y[:], OP.mult)
        nc.vector.tensor_tensor(t[:], t[:], v[:], OP.mult)
        nc.vector.tensor_scalar(t[:], t[:], -0.5, 1.5, OP.mult, OP.add)
        nc.vector.tensor_tensor(y[:], y[:], t[:], OP.mult)
    nc.vector.tensor_copy(out, y[:])


def trace_body(nc, d, out_ap):
    """d: dict name -> DRAM AP (inputs); out_ap: [1,2] f32 DRAM output."""
    # Collective bounce buffers: outputs must be addr_space="Shared" on HW.
    # AllGather (floor ~2x cheaper than AllReduce) + local 8-slot reduce.
    # Stats travel TRANSPOSED ([1, stat*channel]) so each bounce DMA is ONE
    # contiguous packet instead of 128 partitions x 8B (which the DMA engine
    # splits into 16 serialized packets, ~5us per bounce), and as AllReduce
    # so no local 8-slot reduce / gather descramble is needed.
    ar1b = [(nc.dram_tensor(f"ar1i_{l}", [2, 896], F32).ap(),
             nc.dram_tensor(f"ar1o_{l}", [NCORES, 2, 896], F32,
                            addr_space="Shared").ap())
            for l in range(NC)]
    ar2b = [(nc.dram_tensor(f"ar2i_{l}", [1, 256], F32).ap(),
             nc.dram_tensor(f"ar2o_{l}", [NCORES, 256], F32,
                            addr_space="Shared").ap())
            for l in range(NC)]
    with tile.TileContext(nc) as tc:
        with (
            tc.tile_pool(name="big", bufs=1) as big,
            tc.tile_pool(name="cst", bufs=1) as cst,
            tc.tile_pool(name="ph0", bufs=2) as ph0,
            tc.tile_pool(name="gate", bufs=2) as gate,
            tc.tile_pool(name="sm", bufs=2) as sm,
            tc.tile_pool(name="ps_ab", bufs=1, space="PSUM") as ps_ab_p,
            tc.tile_pool(name="ps_st", bufs=1, space="PSUM") as ps_st_p,
            tc.tile_pool(name="ps_f", bufs=3, space="PSUM") as ps_f_p,
            tc.tile_pool(name="ps_c", bufs=2, space="PSUM") as ps_c_p,
            tc.tile_pool(name="ps_m", bufs=1, space="PSUM") as ps_m_p,
        ):
            # ---- load constants (rhs_pack last: only needed by the main
            # matmuls ~40us in; issuing it first would stall the small loads
            # behind a 14us DMA and delay phase0 + the first AllGather) ----
            rhs_pack = big.tile([128, R], BF16)
            atomT = cst.tile([ORIG + 1, BJ], F32)
            nc.sync.dma_start(atomT[:], d["atomT"])
            emb = cst.tile([ORIG + 1, F], F32)
            nc.sync.dma_start(emb[:], d["emb"])
            w3 = cst.tile([K, NC * 128], BF16)
            nc.sync.dma_start(w3[:], d["w3"])
            wab = cst.tile([F + 1, NC * 256], BF16)
            nc.sync.dma_start(wab[:], d["wab"])
            s1s = cst.tile([G32, NC * NBLK * 256], BF16)
            nc.sync.dma_start(s1s[:], d["s1s"])
            aux = cst.tile([G32, 2 * NBLK], BF16)
            nc.sync.dma_start(aux[:], d["aux"])
            ident = cst.tile([128, 128], F32)
            nc.sync.dma_start(ident[:], d["ident"])
            on8 = cst.tile([NCORES, 1], F32)
            nc.vector.memset(on8[:], 1.0)
            sel16 = cst.tile([2 * NCORES, 4], F32)
            nc.sync.dma_start(sel16[:], d["sel16"])
            gvec = cst.tile([128, 12], F32)
            nc.sync.dma_start(gvec[:], d["gvec"])
            gvec2 = cst.tile([F, 6], F32)
            nc.sync.dma_start(gvec2[:], d["gvec2"])
            fcW = cst.tile([F, H], F32)
            nc.sync.dma_start(fcW[:], d["fcW"])
            fcb = cst.tile([H, 1], F32)
            nc.sync.dma_start(fcb[:], d["fcb"])
            outW = cst.tile([H, 1], F32)
            nc.sync.dma_start(outW[:], d["outW"])
            outb = cst.tile([1, 1], F32)
            nc.sync.dma_start(outb[:], d["outb"])
            nc.sync.dma_start(rhs_pack[:], d["rhs_pack"])

            # ---- embedding: fea_ext [65, 192] = [(atom@embW+b)^T ; ones] ----
            # one shared PSUM scratch bank for all small outputs: cols 0:6
            # bn1 stats (transposed back), 16:144 AR2 transpose, 144:146 bn2,
            # 192:384 embedding, 384:386 head fc, 400:401 head out
            ps_misc = ps_m_p.tile([128, 512], F32, tag="pm")
            nc.tensor.matmul(ps_misc[0:F, 192:384], emb[:], atomT[:],
                             start=True, stop=True, skip_group_check=True)
            fea_ext = ph0.tile([F + 1, BJ], F32, tag="fea")
            nc.vector.tensor_copy(fea_ext[0:F, :], ps_misc[0:F, 192:384])
            nc.vector.memset(fea_ext[F:F + 1, :], 1.0)
            # bf16 shadow of fea_ext: keeps the wab matmuls in bf16 mode
            # (fp32 matmul streams at 1/4 rate, 2x LDWEIGHTS)
            feaB = ph0.tile([F + 1, BJ], BF16, tag="feaB")
            nc.vector.tensor_copy(feaB[0:F, :], ps_misc[0:F, 192:384])
            nc.vector.memset(feaB[F:F + 1, :], 1.0)

            for l in range(NC):
                # ================= phase 0: A'/B, lhsT blocks, bn1 stat terms
                lhs_all = ph0.tile([128, NBLK * 2 * F], BF16, tag="lhs")
                nc.vector.memset(lhs_all[:], 0.0)  # rows 41:64 must stay zero
                ps_st = ps_st_p.tile([64, 512], F32, tag="st")
                # ab2 [32, 256] per block = [A'^T | B^T] at base partition 0
                # (TT inputs must share base partition when both in SBUF);
                # kept for all 6 blocks so lhsT assembly can run AFTER the
                # AR1 trigger, inside the collective's latency shadow.
                # ab2_all block b: [A' | B | A'^2 | B^2] at cols 512b; the
                # packed squares let ONE [32,512] matmul cover stats s0-s3
                ab2_all = ph0.tile([G32, NBLK * 512], BF16, tag="ab_sb")
                for b in range(NBLK):
                    o = b * 512
                    # 4 blocks share one psum bank via disjoint col ranges,
                    # so consecutive blocks pipeline without buffer WAR
                    if b % 4 == 0:
                        ps_ab = ps_ab_p.tile([128, 512], F32, tag="ab")
                    po = (b % 4) * 128
                    fsl = feaB[:, b * G32:(b + 1) * G32]
                    nc.tensor.matmul(ps_ab[64:96, po:po + 128], fsl,
                                     wab[:, l * 256:l * 256 + 128],
                                     start=True, stop=True, tile_position=(0, 64),
                                     skip_group_check=True)
                    nc.tensor.matmul(ps_ab[96:128, po:po + 128], fsl,
                                     wab[:, l * 256 + 128:l * 256 + 256],
                                     start=True, stop=True, tile_position=(0, 96),
                                     skip_group_check=True)
                    nc.vector.tensor_copy(ab2_all[:, o:o + 128],
                                          ps_ab[64:96, po:po + 128])
                    nc.vector.tensor_copy(ab2_all[:, o + 128:o + 256],
                                          ps_ab[96:128, po:po + 128])
                    nc.vector.tensor_tensor(ab2_all[:, o + 256:o + 512],
                                            ab2_all[:, o:o + 256],
                                            ab2_all[:, o:o + 256], OP.mult)
                    prod = ph0.tile([G32, 128], BF16, tag="prod")
                    nc.vector.tensor_tensor(prod[:], ab2_all[:, o:o + 128],
                                            ab2_all[:, o + 128:o + 256], OP.mult)
                    crs = ph0.tile([G32, 256], BF16, tag="crs")
                    nc.vector.tensor_tensor(
                        crs[:], ab2_all[:, o:o + 256],
                        s1s[:, (l * NBLK + b) * 256:(l * NBLK + b + 1) * 256],
                        OP.mult)
                    # stat contractions, TRANSPOSED: lhs = [ones|deg] aux
                    # pair; psum row 0 = ones-weighted, row 1 = deg-weighted
                    st, sp_ = (b == 0), (b == NBLK - 1)
                    nc.tensor.matmul(ps_st[0:2, 0:512], aux[:, 2 * b:2 * b + 2],
                                     ab2_all[:, o:o + 512],
                                     start=st, stop=sp_, skip_group_check=True)
                    nc.tensor.matmul(ps_st[32:34, 0:128],
                                     aux[:, 2 * b:2 * b + 2], prod[:],
                                     start=st, stop=sp_, skip_group_check=True)
                    nc.tensor.matmul(ps_st[32:34, 128:384],
                                     aux[:, 2 * b:2 * b + 2], crs[:],
                                     start=st, stop=sp_, skip_group_check=True)

                # ---- AR1: 2-packet bounce + AllGather ----
                ar_sb = sm.tile([2, 896], F32, tag="ar1s")
                nc.vector.tensor_copy(ar_sb[0:2, 0:512], ps_st[0:2, 0:512])
                nc.vector.tensor_copy(ar_sb[0:2, 512:896], ps_st[32:34, 0:384])
                ar_in, ar_out = ar1b[l]
                nc.sync.dma_start(ar_in, ar_sb[:])
                nc.gpsimd.collective_compute(
                    "AllGather", OP.bypass, replica_groups=[list(range(NCORES))],
                    ins=[ar_in], outs=[ar_out])

                # lhsT assembly (W3 + A'/B rows), hidden under AR1 latency
                for b in range(NBLK):
                    o = b * 512
                    fcol, ccol = 2 * b * F, (2 * b + 1) * F
                    nc.vector.tensor_copy(lhs_all[0:K, fcol:fcol + F],
                                          w3[:, l * 128:l * 128 + F])
                    nc.vector.tensor_copy(lhs_all[0:K, ccol:ccol + F],
                                          w3[:, l * 128 + F:(l + 1) * 128])
                    nc.vector.tensor_copy(lhs_all[64:96, fcol:fcol + F],
                                          ab2_all[:, o:o + F])
                    nc.vector.tensor_copy(lhs_all[64:96, ccol:ccol + F],
                                          ab2_all[:, o + F:o + 128])
                    nc.vector.tensor_copy(lhs_all[96:128, fcol:fcol + F],
                                          ab2_all[:, o + 128:o + 128 + F])
                    nc.vector.tensor_copy(lhs_all[96:128, ccol:ccol + F],
                                          ab2_all[:, o + 128 + F:o + 256])

                arR = sm.tile([2 * NCORES, 896], F32, tag="ar1gg")
                nc.sync.dma_start(arR[:], ar_out.rearrange("r s c -> (r s) c"))
                # 8-way reduce + transpose back + bn1 stat weighting in ONE
                # step: sel16 rows select ones/deg wire rows and fold the
                # (N, 1, 2)/NTOT weights, so col0/col1 of psum are the bn1
                # mean / E[x^2] (up to the host-prescaled gvec constants)
                nc.tensor.matmul(ps_misc[:, 0:1], arR[:, 0:128],
                                 sel16[:, 0:1], start=True, stop=False,
                                 skip_group_check=True)
                nc.tensor.matmul(ps_misc[:, 0:1], arR[:, 128:256],
                                 sel16[:, 1:2], start=False, stop=True,
                                 skip_group_check=True)
                nc.tensor.matmul(ps_misc[:, 1:2], arR[:, 256:384],
                                 sel16[:, 0:1], start=True, stop=False,
                                 skip_group_check=True)
                nc.tensor.matmul(ps_misc[:, 1:2], arR[:, 384:512],
                                 sel16[:, 1:2], start=False, stop=False,
                                 skip_group_check=True)
                nc.tensor.matmul(ps_misc[:, 1:2], arR[:, 512:640],
                                 sel16[:, 3:4], start=False, stop=False,
                                 skip_group_check=True)
                nc.tensor.matmul(ps_misc[:, 1:2], arR[:, 640:768],
                                 sel16[:, 2:3], start=False, stop=False,
                                 skip_group_check=True)
                nc.tensor.matmul(ps_misc[:, 1:2], arR[:, 768:896],
                                 sel16[:, 2:3], start=False, stop=True,
                                 skip_group_check=True)
                arg = sm.tile([128, 2], F32, tag="ar1g")
                nc.vector.tensor_copy(arg[:], ps_misc[:, 0:2])

                # bn1 finalize (tiny, f32)
                mean = sm.tile([128, 1], F32, tag="mean")
                nc.vector.tensor_tensor(mean[:], arg[:, 0:1], gvec[:, l:l + 1],
                                        OP.add)
                var = sm.tile([128, 1], F32, tag="var")
                nc.vector.tensor_tensor(var[:], mean[:], mean[:], OP.mult)
                ex2 = sm.tile([128, 1], F32, tag="ex2")
                nc.vector.tensor_tensor(ex2[:], arg[:, 1:2],
                                        gvec[:, 3 + l:4 + l], OP.add)
                nc.vector.tensor_tensor(var[:], ex2[:], var[:], OP.subtract)
                nc.vector.tensor_scalar(var[:], var[:], EPS, None, OP.add)
                inv = sm.tile([128, 1], F32, tag="inv")
                _rsqrt(nc, sm, inv[:], var, 128)
                scl = sm.tile([128, 1], F32, tag="scl")
                nc.vector.tensor_tensor(scl[:], gvec[:, 6 + l:7 + l], inv[:], OP.mult)
                bia = sm.tile([128, 1], F32, tag="bia")
                nc.vector.tensor_tensor(bia[:], mean[:], scl[:], OP.mult)
                nc.vector.tensor_tensor(bia[:], gvec[:, 9 + l:10 + l], bia[:],
                                        OP.subtract)
                # filt-half bn1 affine, duplicated to both partition halves
                sigscl = sm.tile([128, 1], F32, tag="sigscl")
                nc.vector.tensor_copy(sigscl[0:F, :], scl[0:F, :])
                nc.vector.tensor_copy(sigscl[F:128, :], scl[0:F, :])
                sigbia = sm.tile([128, 1], F32, tag="sigbia")
                nc.vector.tensor_copy(sigbia[0:F, :], bia[0:F, :])
                nc.vector.tensor_copy(sigbia[F:128, :], bia[0:F, :])
                spscl = sm.tile([128, 1], F32, tag="spscl")
                nc.vector.tensor_copy(spscl[0:F, :], scl[F:128, :])
                nc.vector.tensor_copy(spscl[F:128, :], scl[F:128, :])
                spbia = sm.tile([128, 1], F32, tag="spbia")
                nc.vector.tensor_copy(spbia[0:F, :], bia[F:128, :])
                nc.vector.tensor_copy(spbia[F:128, :], bia[F:128, :])

                # ---- main matmuls (raw gated) + sig + drain ----
                sig_buf = gate.tile([128, HALF], BF16, tag="sig", bufs=1)
                spin_buf = gate.tile([128, HALF], BF16, tag="spin", bufs=1)
                for g in range(NGRP):
                    bp, j = g // 6, g % 6
                    c0 = bp * 3072 + j * GW
                    c1 = HALF + c0
                    gc = c0
                    ps_f = ps_f_p.tile([128, GW], F32, tag="psf")
                    ps_c = ps_c_p.tile([128, GW], F32, tag="psc")
                    fa = lhs_all[:, 2 * bp * F:(2 * bp + 1) * F]
                    fb = lhs_all[:, 2 * (bp + 3) * F:(2 * (bp + 3) + 1) * F]
                    ca = lhs_all[:, (2 * bp + 1) * F:(2 * bp + 2) * F]
                    cb = lhs_all[:, (2 * (bp + 3) + 1) * F:(2 * (bp + 3) + 2) * F]
                    nc.tensor.matmul(ps_f[0:F, :], fa, rhs_pack[:, c0:c0 + GW],
                                     start=True, stop=True)
                    nc.tensor.matmul(ps_f[F:128, :], fb, rhs_pack[:, c1:c1 + GW],
                                     start=True, stop=True)
                    nc.tensor.matmul(ps_c[0:F, :], ca, rhs_pack[:, c0:c0 + GW],
                                     start=True, stop=True)
                    nc.tensor.matmul(ps_c[F:128, :], cb, rhs_pack[:, c1:c1 + GW],
                                     start=True, stop=True)
                    nc.scalar.activation(sig_buf[:, gc:gc + GW], ps_f[:],
                                         AF.Sigmoid, bias=sigbia[:],
                                         scale=sigscl[:])
                    nc.vector.tensor_scalar(spin_buf[:, gc:gc + GW], ps_c[:],
                                            spscl[:], spbia[:],
                                            OP.mult, OP.add)

                # ---- softplus + mul + k-reduce tree, chunked pipeline ----
                summed = sm.tile([128, 2 * BPC * G32 * 3 // 4], F32, tag="summed")
                # summed [128, 96]: p<64 -> (c, bj 0:96), p>=64 -> (c, bj 96:192)
                # spin holds -(bn1 core affine); Sigmoid+Ln gives -softplus,
                # absorbed by the host-negated bn2 gain. All Sigmoids are
                # emitted before any Ln so the ACT table switches once.
                # tapered chunks: the last (768-col) chunk shortens the
                # serial tail from the final group ACT to the AR2 trigger
                CH_SZ = [2304, 2304, 2304, 1536, 768]
                CH_OFF = [0, 2304, 4608, 6912, 8448]
                s_all = gate.tile([128, HALF], BF16, tag="sall", bufs=1)
                for co, cw in zip(CH_OFF, CH_SZ):
                    nc.scalar.activation(s_all[:, co:co + cw],
                                         spin_buf[:, co:co + cw], AF.Sigmoid)
                for ci, (co, cw) in enumerate(zip(CH_OFF, CH_SZ)):
                    nbj = cw // N
                    so = CH_OFF[ci] // N
                    bn = 2 if cw == 2304 else 1
                    sp_t = gate.tile([128, cw], BF16, tag=f"sp{cw}", bufs=bn)
                    nc.scalar.activation(sp_t[:], s_all[:, co:co + cw], AF.Ln)
                    h_t = gate.tile([128, cw], BF16, tag=f"h{cw}", bufs=bn)
                    nc.vector.tensor_tensor(h_t[:], sig_buf[:, co:co + cw],
                                            sp_t[:], OP.mult)
                    # tree: 96 -> 48 -> 24 -> 12 -> 6 -> 3 -> (2 adds)
                    w = N
                    cur = h_t
                    while w > 3:
                        nw = w // 2
                        nxt = gate.tile([128, nbj * nw], BF16,
                                        tag=f"tr{w}_{cw}", bufs=bn)
                        va = cur[:].rearrange("p (b k) -> p b k", k=w)
                        nc.vector.tensor_tensor(
                            nxt[:].rearrange("p (b k) -> p b k", k=nw),
                            va[:, :, 0:nw], va[:, :, nw:2 * nw], OP.add)
                        cur, w = nxt, nw
                    va = cur[:].rearrange("p (b k) -> p b k", k=3)
                    s01 = gate.tile([128, nbj], F32, tag=f"s01_{cw}", bufs=bn)
                    nc.vector.tensor_tensor(
                        s01[:].rearrange("p (b k) -> p b k", k=1),
                        va[:, :, 0:1], va[:, :, 1:2], OP.add)
                    nc.vector.tensor_tensor(
                        summed[:, so:so + nbj].rearrange(
                            "p (b k) -> p b k", k=1),
                        s01[:].rearrange("p (b k) -> p b k", k=1),
                        va[:, :, 2:3], OP.add)

                # ---- bn2 ----
                NB2 = 2 * BPC * G32 * 3 // 4  # 96
                ar2_sb = sm.tile([128, 2], F32, tag="ar2s")
                nc.vector.tensor_reduce(ar2_sb[:, 0:1], summed[:],
                                        axis=mybir.AxisListType.X, op=OP.add)
                ssq = sm.tile([128, NB2], F32, tag="ssq")
                nc.vector.tensor_tensor(ssq[:], summed[:], summed[:], OP.mult)
                nc.vector.tensor_reduce(ar2_sb[:, 1:2], ssq[:],
                                        axis=mybir.AxisListType.X, op=OP.add)
                # transpose [128,2] -> two [1,128] rows (psum partitions 0
                # and 32) via PE, flatten to one partition so the bounce DMA
                # is a single 1KB packet
                nc.tensor.matmul(ps_misc[0:1, 16:144], ar2_sb[:, 0:1],
                                 ident[:], start=True, stop=True,
                                 skip_group_check=True)
                nc.tensor.matmul(ps_misc[32:33, 16:144], ar2_sb[:, 1:2],
                                 ident[:], start=True, stop=True,
                                 skip_group_check=True)
                ar2T = sm.tile([1, 256], F32, tag="ar2T")
                nc.vector.tensor_copy(ar2T[:, 0:128], ps_misc[0:1, 16:144])
                nc.vector.tensor_copy(ar2T[:, 128:256], ps_misc[32:33, 16:144])
                ar2_in, ar2_out = ar2b[l]
                nc.sync.dma_start(ar2_in, ar2T[:])
                nc.gpsimd.collective_compute(
                    "AllGather", OP.bypass, replica_groups=[list(range(NCORES))],
                    ins=[ar2_in], outs=[ar2_out])
                ar2R = sm.tile([NCORES, 256], F32, tag="ar2gg")
                nc.sync.dma_start(ar2R[:], ar2_out)
                # 8-way reduce + column-half add + transpose back to [64, 2],
                # all via accumulating ones-matmuls
                nc.tensor.matmul(ps_misc[0:F, 144:145], ar2R[:, 0:F], on8[:],
                                 start=True, stop=False, skip_group_check=True)
                nc.tensor.matmul(ps_misc[0:F, 144:145], ar2R[:, F:128], on8[:],
                                 start=False, stop=True, skip_group_check=True)
                nc.tensor.matmul(ps_misc[0:F, 145:146], ar2R[:, 128:128 + F],
                                 on8[:], start=True, stop=False,
                                 skip_group_check=True)
                nc.tensor.matmul(ps_misc[0:F, 145:146], ar2R[:, 128 + F:256],
                                 on8[:], start=False, stop=True,
                                 skip_group_check=True)
                m2 = sm.tile([F, 1], F32, tag="m2")
                nc.vector.tensor_scalar(m2[:], ps_misc[0:F, 144:145],
                                        1.0 / NTOT2, None, OP.mult)
                v2 = sm.tile([F, 1], F32, tag="v2")
                nc.vector.tensor_tensor(v2[:], m2[:], m2[:], OP.mult)
                e2 = sm.tile([F, 1], F32, tag="e2")
                nc.vector.tensor_scalar(e2[:], ps_misc[0:F, 145:146],
                                        1.0 / NTOT2, None, OP.mult)
                nc.vector.tensor_tensor(v2[:], e2[:], v2[:], OP.subtract)
                nc.vector.tensor_scalar(v2[:], v2[:], EPS, None, OP.add)
                i2 = sm.tile([F, 1], F32, tag="i2")
                _rsqrt(nc, sm, i2[:], v2, F)
                s2 = sm.tile([F, 1], F32, tag="s2")
                nc.vector.tensor_tensor(s2[:], gvec2[:, l:l + 1], i2[:], OP.mult)
                b2 = sm.tile([F, 1], F32, tag="b2")
                nc.vector.tensor_tensor(b2[:], m2[:], s2[:], OP.mult)
                nc.vector.tensor_tensor(b2[:], gvec2[:, 3 + l:4 + l], b2[:],
                                        OP.subtract)
                s2d = sm.tile([128, 1], F32, tag="s2d")
                nc.vector.tensor_copy(s2d[0:F, :], s2[:])
                nc.vector.tensor_copy(s2d[F:128, :], s2[:])
                b2d = sm.tile([128, 1], F32, tag="b2d")
                nc.vector.tensor_copy(b2d[0:F, :], b2[:])
                nc.vector.tensor_copy(b2d[F:128, :], b2[:])
                sn = sm.tile([128, NB2], F32, tag="sn")
                nc.vector.tensor_scalar(sn[:], summed[:], s2d[:], b2d[:],
                                        OP.mult, OP.add)
                snc = sm.tile([F, NB2], F32, tag="snc")
                nc.vector.tensor_copy(snc[:], sn[F:128, :])
                tmp = sm.tile([F, BJ], F32, tag="tmpf")
                nc.vector.tensor_tensor(tmp[:, 0:N], sn[0:F, :],
                                        fea_ext[0:F, 0:N], OP.add)
                nc.vector.tensor_tensor(tmp[:, N:BJ], snc[:],
                                        fea_ext[0:F, N:BJ], OP.add)
                fea_new = ph0.tile([F + 1, BJ], F32, tag="fea")
                _softplus(nc, sm, fea_new[0:F, :], tmp[:], "feasp")
                nc.vector.memset(fea_new[F:F + 1, :], 1.0)
                fea_ext = fea_new
                feaB = ph0.tile([F + 1, BJ], BF16, tag="feaB")
                nc.vector.tensor_copy(feaB[0:F, :], fea_new[0:F, :])
                nc.vector.memset(feaB[F:F + 1, :], 1.0)

            # ---- head ----
            crys = sm.tile([F, BPC], F32, tag="crys")
            nc.vector.tensor_reduce(
                crys[:], fea_ext[0:F, :].rearrange("p (a b) -> p a b", b=N),
                axis=mybir.AxisListType.X, op=OP.add)
            nc.vector.tensor_scalar(crys[:], crys[:], 1.0 / N, None, OP.mult)
            crys2 = sm.tile([F, BPC], F32, tag="crys2")
            _softplus(nc, sm, crys2[:], crys[:], "hd1")
            nc.tensor.matmul(ps_misc[:, 384:386], fcW[:], crys2[:],
                             start=True, stop=True, skip_group_check=True)
            sph = sm.tile([H, BPC], F32, tag="sph")
            _softplus(nc, sm, sph[:], ps_misc[:, 384:386], "hd2", nbias=fcb[:])
            nc.tensor.matmul(ps_misc[0:1, 400:402], outW[:], sph[:],
                             start=True, stop=True, skip_group_check=True)
            res = sm.tile([1, BPC], F32, tag="res")
            nc.vector.tensor_scalar(res[:], ps_misc[0:1, 400:402],
                                    outb[0:1, 0:1], None, OP.add)
            nc.sync.dma_start(out_ap, res[:])  # out dram is [1, BPC]
    return nc


# ======================================================================
# Self-contained runner: shard -> compile (cached) -> run SPMD -> gather
# ======================================================================
_COMPILED = {}


def _build_nc():
    import concourse.bacc as bacc
    nc = bacc.Bacc("TRN2", target_bir_lowering=False, debug=False,
                   num_devices=NCORES)
    d = {}
    for name, shape, dt in INPUT_SPECS:
        d[name] = nc.dram_tensor(name, list(shape), dt, kind="ExternalInput").ap()
    out_ap = nc.dram_tensor("out", [1, BPC], F32, kind="ExternalOutput").ap()
    trace_body(nc, d, out_ap)
    nc.compile()
    return nc


def kernel(**inputs):
    from concourse.bass_utils import run_bass_kernel_spmd
    in_maps = host_prep(inputs)
    if "nc" not in _COMPILED:
        _COMPILED["nc"] = _build_nc()
    nc = _COMPILED["nc"]
    res = run_bass_kernel_spmd(nc, in_maps, core_ids=list(range(NCORES)))
    out = np.concatenate([np.asarray(r["out"], np.float32).reshape(BPC)
                          for r in res.results])
    return out.reshape(N0, 1)



# revision 69
# speedup vs baseline: 1.1446x; 1.1446x over previous
"""CrystalGraphConvNet Bass/Tile kernel for TRN2 (8-core data-parallel).

Device algorithm (per core, 2 crystals, BJ=192 bj-rows, R=18432 (bj,k)-rows):
  - gated = conv(total) computed as ONE augmented bf16 matmul per row-block:
      lhsT [128, 64] = [W3 ; 0 ; A'^T_block ; B^T_block], rhs_pack [128, cols] =
      [nbrT ; 0 ; ones-diag ; adj-diag]  -> raw gated in PSUM, partition-packed
      (filt(H0)/filt(H1) stacked to use all 128 lanes downstream).
  - bn1 stats computed analytically (no pass over gated): host supplies
    layer-independent nbr/adj reductions; device computes fea-dependent
    terms TRANSPOSED (aux [ones|deg] as 2-col LDWEIGHTS, data as rhs) so
    the collective payload is [2,896] (2 DMA packets, not 128x8B = 16
    serialized packets); AllGather + a ones/sel16-weighted PE matmul does
    the 8-core reduce, transpose-back to channel-major, AND the bn1
    mean/E[x^2] weighting in one step.
  - sigmoid via ACT Sigmoid table (bn1 affine as per-partition scale/bias
    from PSUM); core half drained on DVE with the NEGATED affine folded in
    (host negates core-half bn1 g/b and bn2 gain), so softplus is
    -Ln(Sigmoid(-z)) and stays in two ACT tables; chunk Sigmoids batched
    before Lns to minimize table loads.
  - h = sig*sp on DVE; k-sum via contiguous-halves add tree (bf16 2x).
  - bn2: free-dim reduce + PE transpose to a [1,256] single-packet payload,
    AllGather + ones-matmul reduce back to [64,2]; fea update via softplus.
"""

import numpy as np
import ml_dtypes

import concourse.bass as bass
import concourse.mybir as mybir
from concourse import tile

F32 = mybir.dt.float32
BF16 = mybir.dt.bfloat16
FP8 = mybir.dt.float8e4
I32 = mybir.dt.int32
AF = mybir.ActivationFunctionType
OP = mybir.AluOpType

EPS = 1e-5
N0, N, ORIG, F, K, H, NC = 16, 96, 92, 64, 41, 128, 3
NCORES, BPC = 8, 2
BJ = BPC * N            # 192
R = BJ * N              # 18432
G32 = 32
NBLK = BJ // G32        # 6
HALF = R // 2           # 9216
NTOT = float(N0 * N * N)
NTOT2 = float(N0 * N)
NGRP = 18               # main groups per layer, 512 paired-cols each
GW = 512
SPCH = 4                # softplus/mul/tree chunks
CHW = HALF // SPCH      # 2304 = 24 bj * 96


def bf16(x):
    return np.ascontiguousarray(np.asarray(x, np.float32).astype(ml_dtypes.bfloat16))


def fp8(x):
    return np.ascontiguousarray(
        np.asarray(x, np.float32).astype(ml_dtypes.float8_e4m3fn))


INPUT_SPECS = [
    ("rhs_pack", (128, R), BF16),
    ("atomT", (ORIG + 1, BJ), F32),
    ("emb", (ORIG + 1, F), F32),
    ("w3", (K, NC * 128), BF16),
    ("wab", (F + 1, NC * 256), BF16),
    ("s1s", (G32, NC * NBLK * 256), BF16),
    ("aux", (G32, 2 * NBLK), BF16),
    ("sel16", (2 * NCORES, 4), F32),
    ("ident", (128, 128), F32),
    ("gvec", (128, 12), F32),
    ("gvec2", (F, 6), F32),
    ("fcW", (F, H), F32),
    ("fcb", (H, 1), F32),
    ("outW", (H, 1), F32),
    ("outb", (1, 1), F32),
]


def host_prep(inputs):
    """Build the 8 per-core input maps from the full problem inputs."""
    atom_fea = np.asarray(inputs["atom_fea"], np.float32)
    nbr_fea = np.asarray(inputs["nbr_fea"], np.float32)
    adj = np.asarray(inputs["adj"])
    conv_W = np.asarray(inputs["conv_W"], np.float64)
    conv_b = np.asarray(inputs["conv_b"], np.float64)

    emb_ext = np.concatenate(
        [np.asarray(inputs["emb_W"], np.float32),
         np.asarray(inputs["emb_b"], np.float32)[None]], 0)
    w3_all = np.concatenate([bf16(conv_W[l, 2 * F:]) for l in range(NC)], 1)
    wab_all = bf16(np.concatenate(
        [np.concatenate(
            [np.concatenate([conv_W[l, :F], conv_b[l][None]], 0),
             np.concatenate([conv_W[l, F:2 * F], np.zeros((1, 2 * F))], 0)], 1)
         for l in range(NC)], 1))
    fcW = np.asarray(inputs["fc_W"], np.float32)
    # negated: consumed as the Sigmoid nbias inside _softplus (see kernel)
    fcb = -np.asarray(inputs["fc_b"], np.float32).reshape(H, 1)
    outW = np.asarray(inputs["out_W"], np.float32).reshape(H, 1)
    outb = np.asarray(inputs["out_b"], np.float32).reshape(1, 1)
    bn1_g = np.asarray(inputs["bn1_g"], np.float32)
    bn1_b = np.asarray(inputs["bn1_b"], np.float32)
    bn2_g = np.asarray(inputs["bn2_g"], np.float32)
    bn2_b = np.asarray(inputs["bn2_b"], np.float32)

    colbj = np.arange(R) // N
    gidx = colbj % G32

    per_core, nbrsum_g, gram_g = [], 0.0, 0.0
    for c in range(NCORES):
        sl = slice(c * BPC, (c + 1) * BPC)
        nbr = nbr_fea[sl].reshape(R, K).astype(np.float64)
        adjf = adj[sl].reshape(R).astype(np.float64)
        deg = adjf.reshape(BJ, N).sum(1)
        rhs = np.zeros((128, R), np.float32)
        rhs[0:K] = nbr.T
        rhs[64 + gidx, np.arange(R)] = 1.0
        rhs[96 + gidx, np.arange(R)] = adjf
        nbrj = nbr.reshape(BJ, N, K).sum(1)
        nbrja = (nbr.reshape(BJ, N, K) * adjf.reshape(BJ, N, 1)).sum(1)
        s1s = np.empty((G32, NC * NBLK * 256), np.float64)
        for l in range(NC):
            W3 = conv_W[l, 2 * F:]
            S1T, S1aT = nbrj @ W3, nbrja @ W3
            for b in range(NBLK):
                blk = np.concatenate(
                    [S1T[b * G32:(b + 1) * G32], S1aT[b * G32:(b + 1) * G32]], 1)
                s1s[:, (l * NBLK + b) * 256:(l * NBLK + b + 1) * 256] = blk
        # per block b: col 2b = ones, col 2b+1 = deg (stat-matmul weights)
        aux = np.zeros((G32, 2 * NBLK), np.float64)
        for b in range(NBLK):
            aux[:, 2 * b] = 1.0
            aux[:, 2 * b + 1] = deg[b * G32:(b + 1) * G32]
        atomT = np.concatenate(
            [atom_fea[sl].reshape(BJ, ORIG).T, np.ones((1, BJ))], 0).astype(np.float32)
        nbrsum_g = nbrsum_g + nbr.sum(0)
        gram_g = gram_g + nbr.T @ nbr
        per_core.append(dict(rhs=bf16(rhs), atomT=atomT, s1s=bf16(s1s), aux=bf16(aux)))

    # Core-half bn1 params and bn2 gain are negated host-side: the kernel
    # computes softplus(z) as -Ln(Sigmoid(-z)), so the core affine must
    # produce -z, and the resulting negated `summed` is fixed up in bn2 by
    # the negated gain (bias formula is sign-invariant).
    # cols 0:3 / 3:6 are pre-divided by NTOT: the AR1 reduce-matmuls also
    # fold the (N, 1, 2)/NTOT stat weights (sel16), so they produce the bn1
    # mean / E[x^2] directly.
    gvec = np.zeros((128, 12), np.float32)
    for l in range(NC):
        W3 = conv_W[l, 2 * F:]
        gvec[:, l] = (nbrsum_g @ W3) / NTOT
        gvec[:, 3 + l] = np.einsum("fc,fg,gc->c", W3, gram_g, W3) / NTOT
        gvec[:, 6 + l] = bn1_g[l]
        gvec[F:128, 6 + l] *= -1.0
        gvec[:, 9 + l] = bn1_b[l]
        gvec[F:128, 9 + l] *= -1.0
    gvec2 = np.zeros((F, 6), np.float32)
    for l in range(NC):
        gvec2[:, l] = -bn2_g[l]
        gvec2[:, 3 + l] = bn2_b[l]

    sel16 = np.zeros((2 * NCORES, 4), np.float32)
    sel16[0::2, 0] = N / NTOT
    sel16[1::2, 1] = 1.0 / NTOT
    sel16[0::2, 2] = 2.0 / NTOT
    sel16[1::2, 3] = 2.0 / NTOT

    in_maps = []
    for c in range(NCORES):
        pc = per_core[c]
        in_maps.append({
            "rhs_pack": pc["rhs"], "atomT": pc["atomT"], "emb": emb_ext,
            "w3": w3_all, "wab": wab_all, "s1s": pc["s1s"], "aux": pc["aux"],
            "ident": np.eye(128, dtype=np.float32), "sel16": sel16,
            "gvec": gvec, "gvec2": gvec2, "fcW": fcW, "fcb": fcb,
            "outW": outW, "outb": outb,
        })
    return in_maps


def _softplus(nc, pool, out, in_, tag, nbias=0.0, nscale=-1.0):
    """out = softplus(x) = Ln(Exp(x) + 1); pass nscale=-scale, nbias=-bias
    (negated args kept for call-site compatibility; Exp flips them back).

    Exp and the chunk-loop Lns can share the natural_log_exp table set,
    so this costs no extra ACT table loads."""
    p, fd = out.shape[0], int(np.prod(out.shape[1:]))
    e = pool.tile([p, fd], F32, tag=tag + "_e")
    if isinstance(nbias, float):
        nc.scalar.activation(e[:], in_, AF.Exp, bias=-nbias, scale=-nscale)
    else:
        nb = pool.tile([p, 1], F32, tag=tag + "_nb")
        nc.vector.tensor_scalar(nb[:], nbias, -1.0, None, OP.mult)
        nc.scalar.activation(e[:], in_, AF.Exp, bias=nb[:], scale=-nscale)
    nc.scalar.activation(out, e[:], AF.Ln, bias=1.0)


def _rsqrt(nc, pool, out, v, p):
    """out = 1/sqrt(v), [p,1] f32, via magic-init + 3 Newton iterations."""
    yb = pool.tile([p, 1], I32, tag="rs_i")
    nc.vector.tensor_scalar(yb[:], v.bitcast(I32), 1, None, OP.logical_shift_right)
    nc.vector.tensor_scalar(yb[:], yb[:], -1, 0x5F3759DF, OP.mult, OP.add)
    y = yb.bitcast(F32)
    t = pool.tile([p, 1], F32, tag="rs_t")
    for _ in range(2):
        nc.vector.tensor_tensor(t[:], y[:], y[:], OP.mult)
        nc.vector.tensor_tensor(t[:], t[:], v[:], OP.mult)
        nc.vector.tensor_scalar(t[:], t[:], -0.5, 1.5, OP.mult, OP.add)
        nc.vector.tensor_tensor(y[:], y[:], t[:], OP.mult)
    nc.vector.tensor_copy(out, y[:])


def trace_body(nc, d, out_ap):
    """d: dict name -> DRAM AP (inputs); out_ap: [1,2] f32 DRAM output."""
    # Collective bounce buffers: outputs must be addr_space="Shared" on HW.
    # AllGather (floor ~2x cheaper than AllReduce) + local 8-slot reduce.
    # Stats travel TRANSPOSED ([1, stat*channel]) so each bounce DMA is ONE
    # contiguous packet instead of 128 partitions x 8B (which the DMA engine
    # splits into 16 serialized packets, ~5us per bounce), and as AllReduce
    # so no local 8-slot reduce / gather descramble is needed.
    ar1b = [(nc.dram_tensor(f"ar1i_{l}", [2, 896], F32).ap(),
             nc.dram_tensor(f"ar1o_{l}", [NCORES, 2, 896], F32,
                            addr_space="Shared").ap())
            for l in range(NC)]
    ar2b = [(nc.dram_tensor(f"ar2i_{l}", [1, 256], F32).ap(),
             nc.dram_tensor(f"ar2o_{l}", [NCORES, 256], F32,
                            addr_space="Shared").ap())
            for l in range(NC)]
    with tile.TileContext(nc) as tc:
        with (
            tc.tile_pool(name="big", bufs=1) as big,
            tc.tile_pool(name="cst", bufs=1) as cst,
            tc.tile_pool(name="ph0", bufs=2) as ph0,
            tc.tile_pool(name="gate", bufs=2) as gate,
            tc.tile_pool(name="sm", bufs=2) as sm,
            tc.tile_pool(name="ps_ab", bufs=1, space="PSUM") as ps_ab_p,
            tc.tile_pool(name="ps_st", bufs=1, space="PSUM") as ps_st_p,
            tc.tile_pool(name="ps_f", bufs=3, space="PSUM") as ps_f_p,
            tc.tile_pool(name="ps_c", bufs=2, space="PSUM") as ps_c_p,
            tc.tile_pool(name="ps_m", bufs=1, space="PSUM") as ps_m_p,
        ):
            # ---- load constants (rhs_pack last: only needed by the main
            # matmuls ~40us in; issuing it first would stall the small loads
            # behind a 14us DMA and delay phase0 + the first AllGather) ----
            rhs_pack = big.tile([128, R], BF16)
            atomT = cst.tile([ORIG + 1, BJ], F32)
            nc.sync.dma_start(atomT[:], d["atomT"])
            emb = cst.tile([ORIG + 1, F], F32)
            nc.sync.dma_start(emb[:], d["emb"])
            w3 = cst.tile([K, NC * 128], BF16)
            nc.sync.dma_start(w3[:], d["w3"])
            wab = cst.tile([F + 1, NC * 256], BF16)
            nc.sync.dma_start(wab[:], d["wab"])
            s1s = cst.tile([G32, NC * NBLK * 256], BF16)
            nc.sync.dma_start(s1s[:], d["s1s"])
            aux = cst.tile([G32, 2 * NBLK], BF16)
            nc.sync.dma_start(aux[:], d["aux"])
            ident = cst.tile([128, 128], F32)
            nc.sync.dma_start(ident[:], d["ident"])
            on8 = cst.tile([NCORES, 1], F32)
            nc.vector.memset(on8[:], 1.0)
            sel16 = cst.tile([2 * NCORES, 4], F32)
            nc.sync.dma_start(sel16[:], d["sel16"])
            gvec = cst.tile([128, 12], F32)
            nc.sync.dma_start(gvec[:], d["gvec"])
            gvec2 = cst.tile([F, 6], F32)
            nc.sync.dma_start(gvec2[:], d["gvec2"])
            fcW = cst.tile([F, H], F32)
            nc.sync.dma_start(fcW[:], d["fcW"])
            fcb = cst.tile([H, 1], F32)
            nc.sync.dma_start(fcb[:], d["fcb"])
            outW = cst.tile([H, 1], F32)
            nc.sync.dma_start(outW[:], d["outW"])
            outb = cst.tile([1, 1], F32)
            nc.sync.dma_start(outb[:], d["outb"])
            nc.sync.dma_start(rhs_pack[:], d["rhs_pack"])

            # ---- embedding: fea_ext [65, 192] = [(atom@embW+b)^T ; ones] ----
            # one shared PSUM scratch bank for all small outputs: cols 0:6
            # bn1 stats (transposed back), 16:144 AR2 transpose, 144:146 bn2,
            # 192:384 embedding, 384:386 head fc, 400:401 head out
            ps_misc = ps_m_p.tile([128, 512], F32, tag="pm")
            nc.tensor.matmul(ps_misc[0:F, 192:384], emb[:], atomT[:],
                             start=True, stop=True, skip_group_check=True)
            fea_ext = ph0.tile([F + 1, BJ], F32, tag="fea")
            nc.vector.tensor_copy(fea_ext[0:F, :], ps_misc[0:F, 192:384])
            nc.vector.memset(fea_ext[F:F + 1, :], 1.0)
            # bf16 shadow of fea_ext: keeps the wab matmuls in bf16 mode
            # (fp32 matmul streams at 1/4 rate, 2x LDWEIGHTS)
            feaB = ph0.tile([F + 1, BJ], BF16, tag="feaB")
            nc.vector.tensor_copy(feaB[0:F, :], ps_misc[0:F, 192:384])
            nc.vector.memset(feaB[F:F + 1, :], 1.0)

            for l in range(NC):
                # ================= phase 0: A'/B, lhsT blocks, bn1 stat terms
                lhs_all = ph0.tile([128, NBLK * 2 * F], BF16, tag="lhs")
                nc.vector.memset(lhs_all[:], 0.0)  # rows 41:64 must stay zero
                ps_st = ps_st_p.tile([64, 512], F32, tag="st")
                # ab2 [32, 256] per block = [A'^T | B^T] at base partition 0
                # (TT inputs must share base partition when both in SBUF);
                # kept for all 6 blocks so lhsT assembly can run AFTER the
                # AR1 trigger, inside the collective's latency shadow.
                # ab2_all block b: [A' | B | A'^2 | B^2] at cols 512b; the
                # packed squares let ONE [32,512] matmul cover stats s0-s3
                ab2_all = ph0.tile([G32, NBLK * 512], BF16, tag="ab_sb")
                for b in range(NBLK):
                    o = b * 512
                    # 4 blocks share one psum bank via disjoint col ranges,
                    # so consecutive blocks pipeline without buffer WAR
                    if b % 4 == 0:
                        ps_ab = ps_ab_p.tile([128, 512], F32, tag="ab")
                    po = (b % 4) * 128
                    fsl = feaB[:, b * G32:(b + 1) * G32]
                    nc.tensor.matmul(ps_ab[64:96, po:po + 128], fsl,
                                     wab[:, l * 256:l * 256 + 128],
                                     start=True, stop=True, tile_position=(0, 64),
                                     skip_group_check=True)
                    nc.tensor.matmul(ps_ab[96:128, po:po + 128], fsl,
                                     wab[:, l * 256 + 128:l * 256 + 256],
                                     start=True, stop=True, tile_position=(0, 96),
                                     skip_group_check=True)
                    nc.vector.tensor_copy(ab2_all[:, o:o + 128],
                                          ps_ab[64:96, po:po + 128])
                    nc.vector.tensor_copy(ab2_all[:, o + 128:o + 256],
                                          ps_ab[96:128, po:po + 128])
                    nc.vector.tensor_tensor(ab2_all[:, o + 256:o + 512],
                                            ab2_all[:, o:o + 256],
                                            ab2_all[:, o:o + 256], OP.mult)
                    prod = ph0.tile([G32, 128], BF16, tag="prod")
                    nc.vector.tensor_tensor(prod[:], ab2_all[:, o:o + 128],
                                            ab2_all[:, o + 128:o + 256], OP.mult)
                    crs = ph0.tile([G32, 256], BF16, tag="crs")
                    nc.vector.tensor_tensor(
                        crs[:], ab2_all[:, o:o + 256],
                        s1s[:, (l * NBLK + b) * 256:(l * NBLK + b + 1) * 256],
                        OP.mult)
                    # stat contractions, TRANSPOSED: lhs = [ones|deg] aux
                    # pair; psum row 0 = ones-weighted, row 1 = deg-weighted
                    st, sp_ = (b == 0), (b == NBLK - 1)
                    nc.tensor.matmul(ps_st[0:2, 0:512], aux[:, 2 * b:2 * b + 2],
                                     ab2_all[:, o:o + 512],
                                     start=st, stop=sp_, skip_group_check=True)
                    nc.tensor.matmul(ps_st[32:34, 0:128],
                                     aux[:, 2 * b:2 * b + 2], prod[:],
                                     start=st, stop=sp_, skip_group_check=True)
                    nc.tensor.matmul(ps_st[32:34, 128:384],
                                     aux[:, 2 * b:2 * b + 2], crs[:],
                                     start=st, stop=sp_, skip_group_check=True)

                # ---- AR1: 2-packet bounce + AllGather ----
                ar_sb = sm.tile([2, 896], F32, tag="ar1s")
                nc.vector.tensor_copy(ar_sb[0:2, 0:512], ps_st[0:2, 0:512])
                nc.vector.tensor_copy(ar_sb[0:2, 512:896], ps_st[32:34, 0:384])
                ar_in, ar_out = ar1b[l]
                nc.sync.dma_start(ar_in, ar_sb[:])
                nc.gpsimd.collective_compute(
                    "AllGather", OP.bypass, replica_groups=[list(range(NCORES))],
                    ins=[ar_in], outs=[ar_out])

                # lhsT assembly (W3 + A'/B rows), hidden under AR1 latency
                for b in range(NBLK):
                    o = b * 512
                    fcol, ccol = 2 * b * F, (2 * b + 1) * F
                    nc.vector.tensor_copy(lhs_all[0:K, fcol:fcol + F],
                                          w3[:, l * 128:l * 128 + F])
                    nc.vector.tensor_copy(lhs_all[0:K, ccol:ccol + F],
                                          w3[:, l * 128 + F:(l + 1) * 128])
                    nc.vector.tensor_copy(lhs_all[64:96, fcol:fcol + F],
                                          ab2_all[:, o:o + F])
                    nc.vector.tensor_copy(lhs_all[64:96, ccol:ccol + F],
                                          ab2_all[:, o + F:o + 128])
                    nc.vector.tensor_copy(lhs_all[96:128, fcol:fcol + F],
                                          ab2_all[:, o + 128:o + 128 + F])
                    nc.vector.tensor_copy(lhs_all[96:128, ccol:ccol + F],
                                          ab2_all[:, o + 128 + F:o + 256])

                arR = sm.tile([2 * NCORES, 896], F32, tag="ar1gg")
                nc.sync.dma_start(arR[:], ar_out.rearrange("r s c -> (r s) c"))
                # 8-way reduce + transpose back + bn1 stat weighting in ONE
                # step: sel16 rows select ones/deg wire rows and fold the
                # (N, 1, 2)/NTOT weights, so col0/col1 of psum are the bn1
                # mean / E[x^2] (up to the host-prescaled gvec constants)
                nc.tensor.matmul(ps_misc[:, 0:1], arR[:, 0:128],
                                 sel16[:, 0:1], start=True, stop=False,
                                 skip_group_check=True)
                nc.tensor.matmul(ps_misc[:, 0:1], arR[:, 128:256],
                                 sel16[:, 1:2], start=False, stop=True,
                                 skip_group_check=True)
                nc.tensor.matmul(ps_misc[:, 1:2], arR[:, 256:384],
                                 sel16[:, 0:1], start=True, stop=False,
                                 skip_group_check=True)
                nc.tensor.matmul(ps_misc[:, 1:2], arR[:, 384:512],
                                 sel16[:, 1:2], start=False, stop=False,
                                 skip_group_check=True)
                nc.tensor.matmul(ps_misc[:, 1:2], arR[:, 512:640],
                                 sel16[:, 3:4], start=False, stop=False,
                                 skip_group_check=True)
                nc.tensor.matmul(ps_misc[:, 1:2], arR[:, 640:768],
                                 sel16[:, 2:3], start=False, stop=False,
                                 skip_group_check=True)
                nc.tensor.matmul(ps_misc[:, 1:2], arR[:, 768:896],
                                 sel16[:, 2:3], start=False, stop=True,
                                 skip_group_check=True)
                arg = sm.tile([128, 2], F32, tag="ar1g")
                nc.vector.tensor_copy(arg[:], ps_misc[:, 0:2])

                # bn1 finalize (tiny, f32)
                mean = sm.tile([128, 1], F32, tag="mean")
                nc.vector.tensor_tensor(mean[:], arg[:, 0:1], gvec[:, l:l + 1],
                                        OP.add)
                var = sm.tile([128, 1], F32, tag="var")
                nc.vector.tensor_tensor(var[:], mean[:], mean[:], OP.mult)
                ex2 = sm.tile([128, 1], F32, tag="ex2")
                nc.vector.tensor_tensor(ex2[:], arg[:, 1:2],
                                        gvec[:, 3 + l:4 + l], OP.add)
                nc.vector.tensor_tensor(var[:], ex2[:], var[:], OP.subtract)
                nc.vector.tensor_scalar(var[:], var[:], EPS, None, OP.add)
                inv = sm.tile([128, 1], F32, tag="inv")
                _rsqrt(nc, sm, inv[:], var, 128)
                scl = sm.tile([128, 1], F32, tag="scl")
                nc.vector.tensor_tensor(scl[:], gvec[:, 6 + l:7 + l], inv[:], OP.mult)
                bia = sm.tile([128, 1], F32, tag="bia")
                nc.vector.tensor_tensor(bia[:], mean[:], scl[:], OP.mult)
                nc.vector.tensor_tensor(bia[:], gvec[:, 9 + l:10 + l], bia[:],
                                        OP.subtract)
                # filt-half bn1 affine, duplicated to both partition halves
                sigscl = sm.tile([128, 1], F32, tag="sigscl")
                nc.vector.tensor_copy(sigscl[0:F, :], scl[0:F, :])
                nc.vector.tensor_copy(sigscl[F:128, :], scl[0:F, :])
                sigbia = sm.tile([128, 1], F32, tag="sigbia")
                nc.vector.tensor_copy(sigbia[0:F, :], bia[0:F, :])
                nc.vector.tensor_copy(sigbia[F:128, :], bia[0:F, :])
                spscl = sm.tile([128, 1], F32, tag="spscl")
                nc.vector.tensor_copy(spscl[0:F, :], scl[F:128, :])
                nc.vector.tensor_copy(spscl[F:128, :], scl[F:128, :])
                spbia = sm.tile([128, 1], F32, tag="spbia")
                nc.vector.tensor_copy(spbia[0:F, :], bia[F:128, :])
                nc.vector.tensor_copy(spbia[F:128, :], bia[F:128, :])

                # ---- main matmuls (raw gated) + sig + drain ----
                sig_buf = gate.tile([128, HALF], BF16, tag="sig", bufs=1)
                spin_buf = gate.tile([128, HALF], BF16, tag="spin", bufs=1)
                for g in range(NGRP):
                    bp, j = g // 6, g % 6
                    c0 = bp * 3072 + j * GW
                    c1 = HALF + c0
                    gc = c0
                    ps_f = ps_f_p.tile([128, GW], F32, tag="psf")
                    ps_c = ps_c_p.tile([128, GW], F32, tag="psc")
                    fa = lhs_all[:, 2 * bp * F:(2 * bp + 1) * F]
                    fb = lhs_all[:, 2 * (bp + 3) * F:(2 * (bp + 3) + 1) * F]
                    ca = lhs_all[:, (2 * bp + 1) * F:(2 * bp + 2) * F]
                    cb = lhs_all[:, (2 * (bp + 3) + 1) * F:(2 * (bp + 3) + 2) * F]
                    nc.tensor.matmul(ps_f[0:F, :], fa, rhs_pack[:, c0:c0 + GW],
                                     start=True, stop=True)
                    nc.tensor.matmul(ps_f[F:128, :], fb, rhs_pack[:, c1:c1 + GW],
                                     start=True, stop=True)
                    nc.tensor.matmul(ps_c[0:F, :], ca, rhs_pack[:, c0:c0 + GW],
                                     start=True, stop=True)
                    nc.tensor.matmul(ps_c[F:128, :], cb, rhs_pack[:, c1:c1 + GW],
                                     start=True, stop=True)
                    nc.scalar.activation(sig_buf[:, gc:gc + GW], ps_f[:],
                                         AF.Sigmoid, bias=sigbia[:],
                                         scale=sigscl[:])
                    nc.vector.tensor_scalar(spin_buf[:, gc:gc + GW], ps_c[:],
                                            spscl[:], spbia[:],
                                            OP.mult, OP.add)

                # ---- softplus + mul + k-reduce tree, chunked pipeline ----
                summed = sm.tile([128, 2 * BPC * G32 * 3 // 4], F32, tag="summed")
                # summed [128, 96]: p<64 -> (c, bj 0:96), p>=64 -> (c, bj 96:192)
                # spin holds -(bn1 core affine); Sigmoid+Ln gives -softplus,
                # absorbed by the host-negated bn2 gain. All Sigmoids are
                # emitted before any Ln so the ACT table switches once.
                s_all = gate.tile([128, HALF], BF16, tag="sall", bufs=1)
                for ch in range(SPCH):
                    co = ch * CHW
                    nc.scalar.activation(s_all[:, co:co + CHW],
                                         spin_buf[:, co:co + CHW], AF.Sigmoid)
                for ch in range(SPCH):
                    co = ch * CHW
                    nbj = CHW // N
                    sp_t = gate.tile([128, CHW], BF16, tag="sp")
                    nc.scalar.activation(sp_t[:], s_all[:, co:co + CHW], AF.Ln)
                    h_t = gate.tile([128, CHW], BF16, tag="h")
                    nc.vector.tensor_tensor(h_t[:], sig_buf[:, co:co + CHW],
                                            sp_t[:], OP.mult)
                    # tree: 96 -> 48 -> 24 -> 12 -> 6 -> 3 -> (2 adds)
                    w = N
                    cur = h_t
                    while w > 3:
                        nw = w // 2
                        nxt = gate.tile([128, nbj * nw], BF16, tag=f"tr{w}")
                        va = cur[:].rearrange("p (b k) -> p b k", k=w)
                        nc.vector.tensor_tensor(
                            nxt[:].rearrange("p (b k) -> p b k", k=nw),
                            va[:, :, 0:nw], va[:, :, nw:2 * nw], OP.add)
                        cur, w = nxt, nw
                    va = cur[:].rearrange("p (b k) -> p b k", k=3)
                    s01 = gate.tile([128, nbj], F32, tag="s01")
                    nc.vector.tensor_tensor(
                        s01[:].rearrange("p (b k) -> p b k", k=1),
                        va[:, :, 0:1], va[:, :, 1:2], OP.add)
                    nc.vector.tensor_tensor(
                        summed[:, ch * nbj:(ch + 1) * nbj].rearrange(
                            "p (b k) -> p b k", k=1),
                        s01[:].rearrange("p (b k) -> p b k", k=1),
                        va[:, :, 2:3], OP.add)

                # ---- bn2 ----
                NB2 = 2 * BPC * G32 * 3 // 4  # 96
                ar2_sb = sm.tile([128, 2], F32, tag="ar2s")
                nc.vector.tensor_reduce(ar2_sb[:, 0:1], summed[:],
                                        axis=mybir.AxisListType.X, op=OP.add)
                ssq = sm.tile([128, NB2], F32, tag="ssq")
                nc.vector.tensor_tensor(ssq[:], summed[:], summed[:], OP.mult)
                nc.vector.tensor_reduce(ar2_sb[:, 1:2], ssq[:],
                                        axis=mybir.AxisListType.X, op=OP.add)
                # transpose [128,2] -> two [1,128] rows (psum partitions 0
                # and 32) via PE, flatten to one partition so the bounce DMA
                # is a single 1KB packet
                nc.tensor.matmul(ps_misc[0:1, 16:144], ar2_sb[:, 0:1],
                                 ident[:], start=True, stop=True,
                                 skip_group_check=True)
                nc.tensor.matmul(ps_misc[32:33, 16:144], ar2_sb[:, 1:2],
                                 ident[:], start=True, stop=True,
                                 skip_group_check=True)
                ar2T = sm.tile([1, 256], F32, tag="ar2T")
                nc.vector.tensor_copy(ar2T[:, 0:128], ps_misc[0:1, 16:144])
                nc.vector.tensor_copy(ar2T[:, 128:256], ps_misc[32:33, 16:144])
                ar2_in, ar2_out = ar2b[l]
                nc.sync.dma_start(ar2_in, ar2T[:])
                nc.gpsimd.collective_compute(
                    "AllGather", OP.bypass, replica_groups=[list(range(NCORES))],
                    ins=[ar2_in], outs=[ar2_out])
                ar2R = sm.tile([NCORES, 256], F32, tag="ar2gg")
                nc.sync.dma_start(ar2R[:], ar2_out)
                # 8-way reduce + column-half add + transpose back to [64, 2],
                # all via accumulating ones-matmuls
                nc.tensor.matmul(ps_misc[0:F, 144:145], ar2R[:, 0:F], on8[:],
                                 start=True, stop=False, skip_group_check=True)
                nc.tensor.matmul(ps_misc[0:F, 144:145], ar2R[:, F:128], on8[:],
                                 start=False, stop=True, skip_group_check=True)
                nc.tensor.matmul(ps_misc[0:F, 145:146], ar2R[:, 128:128 + F],
                                 on8[:], start=True, stop=False,
                                 skip_group_check=True)
                nc.tensor.matmul(ps_misc[0:F, 145:146], ar2R[:, 128 + F:256],
                                 on8[:], start=False, stop=True,
                                 skip_group_check=True)
                m2 = sm.tile([F, 1], F32, tag="m2")
                nc.vector.tensor_scalar(m2[:], ps_misc[0:F, 144:145],
                                        1.0 / NTOT2, None, OP.mult)
                v2 = sm.tile([F, 1], F32, tag="v2")
                nc.vector.tensor_tensor(v2[:], m2[:], m2[:], OP.mult)
                e2 = sm.tile([F, 1], F32, tag="e2")
                nc.vector.tensor_scalar(e2[:], ps_misc[0:F, 145:146],
                                        1.0 / NTOT2, None, OP.mult)
                nc.vector.tensor_tensor(v2[:], e2[:], v2[:], OP.subtract)
                nc.vector.tensor_scalar(v2[:], v2[:], EPS, None, OP.add)
                i2 = sm.tile([F, 1], F32, tag="i2")
                _rsqrt(nc, sm, i2[:], v2, F)
                s2 = sm.tile([F, 1], F32, tag="s2")
                nc.vector.tensor_tensor(s2[:], gvec2[:, l:l + 1], i2[:], OP.mult)
                b2 = sm.tile([F, 1], F32, tag="b2")
                nc.vector.tensor_tensor(b2[:], m2[:], s2[:], OP.mult)
                nc.vector.tensor_tensor(b2[:], gvec2[:, 3 + l:4 + l], b2[:],
                                        OP.subtract)
                s2d = sm.tile([128, 1], F32, tag="s2d")
                nc.vector.tensor_copy(s2d[0:F, :], s2[:])
                nc.vector.tensor_copy(s2d[F:128, :], s2[:])
                b2d = sm.tile([128, 1], F32, tag="b2d")
                nc.vector.tensor_copy(b2d[0:F, :], b2[:])
                nc.vector.tensor_copy(b2d[F:128, :], b2[:])
                sn = sm.tile([128, NB2], F32, tag="sn")
                nc.vector.tensor_scalar(sn[:], summed[:], s2d[:], b2d[:],
                                        OP.mult, OP.add)
                snc = sm.tile([F, NB2], F32, tag="snc")
                nc.vector.tensor_copy(snc[:], sn[F:128, :])
                tmp = sm.tile([F, BJ], F32, tag="tmpf")
                nc.vector.tensor_tensor(tmp[:, 0:N], sn[0:F, :],
                                        fea_ext[0:F, 0:N], OP.add)
                nc.vector.tensor_tensor(tmp[:, N:BJ], snc[:],
                                        fea_ext[0:F, N:BJ], OP.add)
                fea_new = ph0.tile([F + 1, BJ], F32, tag="fea")
                _softplus(nc, sm, fea_new[0:F, :], tmp[:], "feasp")
                nc.vector.memset(fea_new[F:F + 1, :], 1.0)
                fea_ext = fea_new
                feaB = ph0.tile([F + 1, BJ], BF16, tag="feaB")
                nc.vector.tensor_copy(feaB[0:F, :], fea_new[0:F, :])
                nc.vector.memset(feaB[F:F + 1, :], 1.0)

            # ---- head ----
            crys = sm.tile([F, BPC], F32, tag="crys")
            nc.vector.tensor_reduce(
                crys[:], fea_ext[0:F, :].rearrange("p (a b) -> p a b", b=N),
                axis=mybir.AxisListType.X, op=OP.add)
            nc.vector.tensor_scalar(crys[:], crys[:], 1.0 / N, None, OP.mult)
            crys2 = sm.tile([F, BPC], F32, tag="crys2")
            _softplus(nc, sm, crys2[:], crys[:], "hd1")
            nc.tensor.matmul(ps_misc[:, 384:386], fcW[:], crys2[:],
                             start=True, stop=True, skip_group_check=True)
            sph = sm.tile([H, BPC], F32, tag="sph")
            _softplus(nc, sm, sph[:], ps_misc[:, 384:386], "hd2", nbias=fcb[:])
            nc.tensor.matmul(ps_misc[0:1, 400:402], outW[:], sph[:],
                             start=True, stop=True, skip_group_check=True)
            res = sm.tile([1, BPC], F32, tag="res")
            nc.vector.tensor_scalar(res[:], ps_misc[0:1, 400:402],
                                    outb[0:1, 0:1], None, OP.add)
            nc.sync.dma_start(out_ap, res[:])  # out dram is [1, BPC]
    return nc


# ======================================================================
# Self-contained runner: shard -> compile (cached) -> run SPMD -> gather
# ======================================================================
_COMPILED = {}


def _build_nc():
    import concourse.bacc as bacc
    nc = bacc.Bacc("TRN2", target_bir_lowering=False, debug=False,
                   num_devices=NCORES)
    d = {}
    for name, shape, dt in INPUT_SPECS:
        d[name] = nc.dram_tensor(name, list(shape), dt, kind="ExternalInput").ap()
    out_ap = nc.dram_tensor("out", [1, BPC], F32, kind="ExternalOutput").ap()
    trace_body(nc, d, out_ap)
    nc.compile()
    return nc


def kernel(**inputs):
    from concourse.bass_utils import run_bass_kernel_spmd
    in_maps = host_prep(inputs)
    if "nc" not in _COMPILED:
        _COMPILED["nc"] = _build_nc()
    nc = _COMPILED["nc"]
    res = run_bass_kernel_spmd(nc, in_maps, core_ids=list(range(NCORES)))
    out = np.concatenate([np.asarray(r["out"], np.float32).reshape(BPC)
                          for r in res.results])
    return out.reshape(N0, 1)

